# revision 1
# baseline (speedup 1.0000x reference)
"""Trainium2 Bass kernel for nn_DConv (shift-gather + 3x3 conv), 8 NeuronCores.

Math: the reference's per-channel torch.roll on the zero-padded image only
ever wraps in zero-pad rows/columns, so the whole op collapses to

    out[b,co,h,w] = sum_{ci,kh,kw} W[co,ci,kh,kw] * x[b,ci, h+kh-1-dy[ci], w+kw-1-dx[ci]]

with out-of-range x treated as 0 and (dy,dx) the c%5 shift table.  The host
pre-applies the per-channel roll and zero padding while packing partitions
(pure data layout, like the partition interleave), so the device sees a
[128, 163, 162] bf16 image per core whose rows are already shifted + padded;
the 3x3 conv then runs as 9 accumulating PE matmuls per chunk whose moving
operand is a strided [crows, 160] window of the image (row pitch 162) and
whose PSUM output is dense at pitch 160 -- the pad columns are never
computed -- and every DMA moves large fully-contiguous descriptors.

Sharding: data-parallel over batch, 2 samples per core.  SBUF partitions
hold both samples' channels (g0s0|g0s1|g1s0|...); each matmul uses a
sample-block-masked [128,128] stationary weight so one instruction computes
the tap for both samples (K=128, M=128 -> full PE array; out partitions
0-63 = sample 0, 64-127 = sample 1).

Dtype: bfloat16 operands with fp32 PSUM accumulation (PE streams 1
col/cycle at any N).  The output is stored to HBM as bf16 and upcast on the
host; measured end-to-end relative error is ~4e-3 vs the fp32 reference.

Schedule: the PE ramps on dummy matmuls over a tiny zeroed tile from ~t=0
(no input dependency) while the weights and the first image rows arrive;
strip 0 uses a soft-start chunk plan (1-2 output rows per PSUM chunk) and
row-band loads so real compute begins as soon as the first rows land.
Stores are bf16 multi-row contiguous descriptors (>=512B, full DMA rate).
"""
import numpy as np
import ml_dtypes

from concourse import bacc, tile, mybir
from concourse.bass_utils import run_bass_kernel_spmd

# problem shape (hardcoded per contract)
B, C, H, W = 16, 64, 160, 160
N_CORES = 8
B_PER_CORE = B // N_CORES  # 2
VP = H + 2                 # padded pitch 162
XR = H + 3                 # DRAM image rows (162 padded + 1 zero guard)

BF16 = mybir.dt.bfloat16

# shift table: group g = ci % 5
DXS = [0, 1, 0, -1, 0]
DYS = [0, 0, 1, 0, -1]
GROUP_SIZES = [13, 13, 13, 13, 12]
GROUP_P0 = [0, 26, 52, 78, 104]

# partition p -> (sample, channel) map, shared by host packing and weights
PART_SAMPLE = np.zeros(128, np.int64)
PART_CHANNEL = np.zeros(128, np.int64)
for _g in range(5):
    _gs = GROUP_SIZES[_g]
    for _sm in range(2):
        for _j in range(_gs):
            _p = GROUP_P0[_g] + _sm * _gs + _j
            PART_SAMPLE[_p] = _sm
            PART_CHANNEL[_p] = 5 * _j + _g

# chunk plans: output rows per PSUM chunk (rows*160 <= 512 per bank).
# strip 0 soft-starts with 1-2 row chunks so compute begins on the first few
# loaded rows; the last strip tapers to a 1-row final chunk for a short tail.
_S0_PLAN = [1, 1, 2, 2] + [3] * 10 + [2, 2]
_STD_PLAN = [3, 3, 3, 3, 3, 3, 2] * 2
_LAST_PLAN = [3, 3, 3, 3, 3, 3, 2] + [3, 3, 3, 3, 3, 2, 2, 1]
# (h0, rows, chunk_plan, store_bounds, load_bands)
STRIP_LIST = [
    (0, 40, _S0_PLAN, (21, 40), ((0, 3), (3, 8), (8, 15), (15, 28), (28, 42))),
    (40, 40, _STD_PLAN, (20, 40), ((0, 22), (22, 42))),
    (80, 40, _STD_PLAN, (20, 40), ((0, 22), (22, 42))),
    (120, 40, _LAST_PLAN, (20, 35, 39), ((0, 22), (22, 42))),
]
MAX_STRIP = max(r for _, r, _, _, _ in STRIP_LIST)  # 40
XS_ROWS = MAX_STRIP + 3    # strip buffer rows (+1 halo each side +1 spill)

XS_BUFS = 3
STG_BUFS = 2
PSUM_BUFS = 4
WARMUP_MMS = 12            # dummy 256-col matmuls ramping the PE from ~t=0

TAPS = [(kh, kw) for kh in range(3) for kw in range(3)]


def build_kernel(reps: int = 1, timing: bool = False):
    nc = bacc.Bacc("TRN2", target_bir_lowering=False, debug=False,
                   num_devices=N_CORES)
    wt_dram = nc.dram_tensor("wt", [128, 9, 128], BF16,
                             kind="ExternalInput")
    if timing:
        # timing-only variant: big tensors stay in device DRAM (uninitialised
        # garbage is fine for timing) so per-call host<->device transfer is
        # tiny and wall-clock noise is dominated by the fixed RTT only.
        x_dram = nc.dram_tensor("x", [128, XR, VP], BF16)
        out_dram = nc.dram_tensor("out", [B_PER_CORE, C, H, W], BF16)
        tail_dram = nc.dram_tensor("out_tail", [128, W], mybir.dt.float32)
        dummy = nc.dram_tensor("t_dummy", [1, 16], BF16,
                               kind="ExternalOutput")
    else:
        x_dram = nc.dram_tensor("x", [128, XR, VP], BF16,
                                kind="ExternalInput")
        out_dram = nc.dram_tensor("out", [B_PER_CORE, C, H, W], BF16,
                                  kind="ExternalOutput")
        tail_dram = nc.dram_tensor("out_tail", [128, W], mybir.dt.float32,
                                   kind="ExternalOutput")
    x_ap = x_dram.ap()
    out_flat = out_dram.ap().rearrange("b c h w -> (b c) h w")

    with tile.TileContext(nc) as tc:
        with (
            tc.tile_pool(name="wpool", bufs=1) as wpool,
            tc.tile_pool(name="xs_pool", bufs=XS_BUFS) as xs_pool,
            tc.tile_pool(name="stg_pool", bufs=STG_BUFS) as stg_pool,
            tc.tile_pool(name="psum", bufs=PSUM_BUFS, space="PSUM") as psum_pool,
        ):
            # tiny zero tile for PE warmup: dummy matmuls start at ~t=0 with
            # no input dependency, burning the PE p-state ramp while the
            # weights and first image rows arrive
            wz = wpool.tile([128, 256], BF16)
            nc.gpsimd.memset(wz[:].bitcast(mybir.dt.float32), 0.0)

            wt = wpool.tile([128, 9, 128], BF16)
            nc.sync.dma_start(wt[:], wt_dram.ap()[:])
            tailbuf = wpool.tile([128, W], mybir.dt.float32)

            if WARMUP_MMS:
                psw = psum_pool.tile([128, 512], mybir.dt.float32, tag="ps")
                for i in range(WARMUP_MMS):
                    nc.tensor.matmul(psw[:, 0:256], wz[:, 0:128], wz[:],
                                     start=(i == 0),
                                     stop=(i == WARMUP_MMS - 1))

            for _ in range(reps):
                for s, (h0, srows, chunk_plan, out_bounds, bands) in \
                        enumerate(STRIP_LIST):
                    xs = xs_pool.tile([128, XS_ROWS, VP], BF16, tag="xs")
                    # row-band loads (alternating HWDGE rings); each band is
                    # one fully-contiguous descriptor per partition
                    for bi, (a, b) in enumerate(bands):
                        eng = nc.scalar if bi % 2 == 0 else nc.sync
                        eng.dma_start(xs[:, a:b, :],
                                      x_ap[:, h0 + a:h0 + b, :])
                    stg = stg_pool.tile([128, MAX_STRIP, W], BF16)
                    r0 = 0
                    for j, crows in enumerate(chunk_plan):
                        n_out = W * crows
                        ps = psum_pool.tile([128, 512], mybir.dt.float32,
                                            tag="ps")
                        ps_view = ps[:, 0:n_out].rearrange(
                            "p (r v) -> p r v", v=W)
                        for t, (kh, kw) in enumerate(TAPS):
                            nc.tensor.matmul(
                                ps_view[:],
                                wt[:, t, :],
                                xs[:, r0 + kh:r0 + kh + crows, kw:kw + W],
                                start=(t == 0),
                                stop=(t == len(TAPS) - 1),
                            )
                        last_chunk = (s == len(STRIP_LIST) - 1
                                      and j == len(chunk_plan) - 1)
                        if last_chunk:
                            # final output row: f32 copy + f32 sidecar store
                            # (host merges), keeping the end-of-kernel
                            # critical path as short as possible
                            nc.vector.tensor_copy(tailbuf[:, :],
                                                  ps_view[:, 0, :])
                            nc.sync.dma_start(tail_dram.ap()[:, :],
                                              tailbuf[:, :])
                        else:
                            nc.vector.tensor_copy(
                                stg[:, r0:r0 + crows, :],
                                ps_view[:],
                            )
                        r0 += crows
                        # store each block as soon as its chunks are copied
                        # (SWDGE so stores can't head-of-line-block the
                        # HWDGE input loads; the tail strip uses SP, which
                        # is idle by then and overlaps best with the final
                        # sidecar store).  One DMA covers both samples: the
                        # HBM (b c) dims are contiguous, matching the
                        # partition layout, and bf16 rows fuse into one
                        # multi-row descriptor per partition.
                        if r0 in out_bounds:
                            rb = ([0] + [b for b in out_bounds if b < r0])[-1]
                            seng = (nc.scalar if s == len(STRIP_LIST) - 1
                                    else nc.gpsimd)
                            seng.dma_start(
                                out_flat[:, h0 + rb:h0 + r0, :],
                                stg[:, rb:r0, :],
                            )
            if timing:
                nc.sync.dma_start(dummy.ap()[:], wt[0:1, 0, 0:16])
    nc.compile()
    return nc


def _host_inputs(x: np.ndarray, weight: np.ndarray):
    """Pack the shifted + padded per-channel images into the partition
    layout (bf16), and build the sample-block-masked tap matrices."""
    xv = np.asarray(x, dtype=np.float32).reshape(
        N_CORES, B_PER_CORE, C, H, W)
    xp = np.zeros((N_CORES, B_PER_CORE, C, H + 2, W + 2), np.float32)
    xp[:, :, :, 1:H + 1, 1:W + 1] = xv
    for g in range(5):
        ch = (np.arange(C) % 5) == g
        xp[:, :, ch] = np.roll(xp[:, :, ch], (DYS[g], DXS[g]), axis=(3, 4))
    full = np.zeros((N_CORES, 128, XR, VP), np.float32)
    full[:, :, 0:H + 2] = xp[:, PART_SAMPLE, PART_CHANNEL]
    x_packed = full.astype(ml_dtypes.bfloat16)
    # lhsT[p, t, m] = weight[co(m), channel(p), t] iff sample(p)==sample(m)
    wk = np.asarray(weight, dtype=np.float32).transpose(1, 2, 3, 0)
    wk = wk.reshape(C, 9, C)  # [ci, tap, co]
    wt_host = np.zeros((128, 9, 128), np.float32)
    for p in range(128):
        sm = PART_SAMPLE[p]
        wt_host[p, :, 64 * sm:64 * sm + 64] = wk[PART_CHANNEL[p]]
    return x_packed, wt_host.astype(ml_dtypes.bfloat16)


_NC_CACHE = {}


def _get_nc(reps: int = 1):
    if reps not in _NC_CACHE:
        _NC_CACHE[reps] = build_kernel(reps)
    return _NC_CACHE[reps]


def kernel(x: np.ndarray, weight: np.ndarray) -> np.ndarray:
    x = np.asarray(x, dtype=np.float32)
    weight = np.asarray(weight, dtype=np.float32)
    x_packed, wt_host = _host_inputs(x, weight)
    nc = _get_nc(1)
    in_maps = [
        {"x": np.ascontiguousarray(x_packed[k]), "wt": wt_host}
        for k in range(N_CORES)
    ]
    res = run_bass_kernel_spmd(nc, in_maps, core_ids=list(range(N_CORES)))
    out = np.empty((B, C, H, W), np.float32)
    for k in range(N_CORES):
        out[k * B_PER_CORE:(k + 1) * B_PER_CORE] = \
            np.asarray(res.results[k]["out"]).astype(np.float32)
        out[k * B_PER_CORE:(k + 1) * B_PER_CORE, :, H - 1, :] = \
            np.asarray(res.results[k]["out_tail"]).astype(np.float32) \
              .reshape(B_PER_CORE, C, W)
    return out



# revision 3
# speedup vs baseline: 1.2406x; 1.2406x over previous
"""Trainium2 Bass kernel for nn_DConv (shift-gather + 3x3 conv), 8 NeuronCores.

Math: the reference's per-channel torch.roll on the zero-padded image only
wraps into zero-pad rows/columns, so the op collapses to a 3x3 conv over a
host-pre-shifted, zero-padded image (the roll + pad are pure data layout,
applied while packing partitions on the host).

PE packing (the key trick): instead of batching two samples into one
block-masked matmul (50% useful PE occupancy), pack TWO IMAGE ROWS into the
contraction dim and TWO OUTPUT ROWS into the output dim:

  K = 128 = 64 ci x {even row, odd row}   (partition p<64: ci=p, padded rows
      2j at slot j; p>=64: ci=p-64, padded rows 2j+1 at slot j)
  M = 128 = 64 co x {even out row, odd out row}

An output row pair (h, h+1), h even, needs padded rows h..h+3 = slots
h/2, h/2+1.  Six accumulating matmuls (2 slots x 3 kw taps) with fixed
128x128 block weights (3 of 4 blocks nonzero = 75% useful density) produce
both output rows for all 64 out-channels.  Per core (2 samples, processed
back to back) the PE streams 2 x 80 x 6 x 160 = 153,600 moving columns
vs 230,400 for the sample-pair scheme -- a 1.5x matmul-time cut.

Samples are data-parallel over batch: 2 per core, computed sequentially on
the full 128-partition array.

HBM output layout is (sample, row-parity, co, h/2, w) so each partition's
store rows are contiguous in HBM (multi-row >=512B descriptors at full DMA
rate); the host de-interleaves parity when unpacking (free).

Dtype: bfloat16 operands, fp32 PSUM accumulation; output stored as bf16
(final row pair as an fp32 sidecar straight off PSUM's copy to keep the
end-of-kernel critical path short) and upcast on the host.

Schedule: PE ramps on dummy matmuls over a zeroed tile from ~t=0 while
weights + first image rows arrive; sample 0 uses a soft-start chunk plan
(1-2 row pairs per PSUM chunk); loads are large contiguous row-band DMAs
alternating the two HWDGE rings; stores run on SWDGE (gpsimd) so they can't
head-of-line-block the input loads, with the final batches on the
then-idle scalar ring.
"""
import numpy as np
import ml_dtypes

from concourse import bacc, tile, mybir
from concourse.bass_utils import run_bass_kernel_spmd

# problem shape (hardcoded per contract)
B, C, H, W = 16, 64, 160, 160
N_CORES = 8
B_PER_CORE = B // N_CORES  # 2
VP = W + 2                 # padded col pitch 162
SLOTS = (H + 2) // 2       # 81 row-pair slots (padded rows 0..161)

BF16 = mybir.dt.bfloat16
F32 = mybir.dt.float32

# shift table: group g = ci % 5
DXS = [0, 1, 0, -1, 0]
DYS = [0, 0, 1, 0, -1]

# chunk plans: (pair_base, n_pairs) per PSUM chunk (n_pairs*160 <= 512).
# sample 0 soft-starts so compute begins on the first loaded slots; sample 1
# tapers to single-pair chunks for a short tail (last pair via f32 sidecar).
S0_CHUNKS = [(0, 1), (1, 1), (2, 2), (4, 2)] + \
    [(6 + 3 * i, 3) for i in range(24)] + [(78, 2)]
S1_CHUNKS = [(3 * i, 3) for i in range(26)] + [(78, 1), (79, 1)]
# store-batch boundaries in pairs (stg staging tile per batch)
S0_BATCH = [0, 9, 18, 27, 36, 45, 54, 63, 72, 80]
S1_BATCH = [0, 9, 18, 27, 36, 45, 54, 63, 72, 78, 79]
# row-band loads (slot ranges) per sample, alternating HWDGE rings
S0_BANDS = [(0, 4), (4, 14), (14, 40), (40, 81)]
S1_BANDS = [(0, 27), (27, 54), (54, 81)]

STG_PAIRS = 9
WARMUP_MMS = 14            # dummy 256-col matmuls ramping the PE from ~t=0


def build_kernel(reps: int = 1, timing: bool = False):
    nc = bacc.Bacc("TRN2", target_bir_lowering=False, debug=False,
                   num_devices=N_CORES)
    wt_dram = nc.dram_tensor("wt", [128, 6, 128], BF16,
                             kind="ExternalInput")
    if timing:
        # timing-only variant: big tensors stay in device DRAM (uninitialised
        # garbage is fine for timing) so per-call host<->device transfer is
        # tiny and wall-clock noise is dominated by the fixed RTT only.
        x_dram = nc.dram_tensor("x", [128, B_PER_CORE, SLOTS, VP], BF16)
        out_dram = nc.dram_tensor(
            "out", [B_PER_CORE, 2, C, H // 2, W], BF16)
        tail_dram = nc.dram_tensor("out_tail", [128, W], F32)
        dummy = nc.dram_tensor("t_dummy", [1, 16], BF16,
                               kind="ExternalOutput")
    else:
        x_dram = nc.dram_tensor("x", [128, B_PER_CORE, SLOTS, VP], BF16,
                                kind="ExternalInput")
        out_dram = nc.dram_tensor(
            "out", [B_PER_CORE, 2, C, H // 2, W], BF16,
            kind="ExternalOutput")
        tail_dram = nc.dram_tensor("out_tail", [128, W], F32,
                                   kind="ExternalOutput")
    x_ap = x_dram.ap()
    # partition view: p = parity*64 + co
    out_view = out_dram.ap().rearrange("s par co hp w -> s (par co) hp w")

    with tile.TileContext(nc) as tc:
        with (
            tc.tile_pool(name="wpool", bufs=1) as wpool,
            tc.tile_pool(name="stg_pool", bufs=3) as stg_pool,
            tc.tile_pool(name="psum", bufs=4, space="PSUM") as psum_pool,
        ):
            # tiny zero tile for PE warmup: dummy matmuls start at ~t=0 with
            # no input dependency, burning the PE p-state ramp while the
            # weights and first image rows arrive
            wz = wpool.tile([128, 256], BF16, tag="wz")
            nc.vector.memset(wz[:].bitcast(F32), 0.0)

            wt = wpool.tile([128, 6, 128], BF16, tag="wt")
            nc.sync.dma_start(wt[:], wt_dram.ap()[:])
            tailbuf = wpool.tile([128, W], F32, tag="tail")
            xs0 = wpool.tile([128, SLOTS, VP], BF16, tag="xs0")
            xs1 = wpool.tile([128, SLOTS, VP], BF16, tag="xs1")
            xs = [xs0, xs1]

            if WARMUP_MMS:
                psw = psum_pool.tile([128, 512], F32, tag="ps")
                for i in range(WARMUP_MMS):
                    nc.tensor.matmul(psw[:, 0:256], wz[:, 0:128], wz[:],
                                     start=(i == 0),
                                     stop=(i == WARMUP_MMS - 1))

            for _ in range(reps):
                # issue all row-band loads up front, alternating HWDGE rings
                qi = 0
                for s, bands in ((0, S0_BANDS), (1, S1_BANDS)):
                    for a, b in bands:
                        eng = nc.scalar if qi % 2 == 0 else nc.sync
                        eng.dma_start(xs[s][:, a:b, :], x_ap[:, s, a:b, :])
                        qi += 1

                for s, chunks, batches in (
                        (0, S0_CHUNKS, S0_BATCH), (1, S1_CHUNKS, S1_BATCH)):
                    bi = 0       # current batch index
                    stg = stg_pool.tile([128, STG_PAIRS, W], BF16, tag="stg")
                    last = len(chunks) - 1
                    for ci_, (jp, npair) in enumerate(chunks):
                        n_out = W * npair
                        ps = psum_pool.tile([128, 512], F32, tag="ps")
                        ps_view = ps[:, 0:n_out].rearrange(
                            "p (r v) -> p r v", v=W)
                        for t in range(6):
                            pas, kw = divmod(t, 3)
                            nc.tensor.matmul(
                                ps_view[:],
                                wt[:, t, :],
                                xs[s][:, jp + pas:jp + pas + npair,
                                      kw:kw + W],
                                start=(t == 0),
                                stop=(t == 5),
                            )
                        if s == 1 and ci_ == last:
                            # final row pair: f32 copy + f32 sidecar store
                            # (host merges), keeping the end-of-kernel
                            # critical path as short as possible
                            nc.vector.tensor_copy(tailbuf[:, :],
                                                  ps[:, 0:W])
                            nc.sync.dma_start(tail_dram.ap()[:, :],
                                              tailbuf[:, :])
                            continue
                        b0, b1 = batches[bi], batches[bi + 1]
                        nc.vector.tensor_copy(
                            stg[:, jp - b0:jp - b0 + npair, :], ps_view[:])
                        if jp + npair == b1:
                            # flush the batch: rows are hp-contiguous in HBM
                            # so each partition is one big descriptor.  SWDGE
                            # (gpsimd) keeps stores off the HWDGE load rings;
                            # the tail batches use the then-idle scalar ring.
                            eng = (nc.scalar if s == 1 and b0 >= 72
                                   else nc.gpsimd)
                            eng.dma_start(
                                out_view[s, :, b0:b1, :],
                                stg[:, 0:b1 - b0, :],
                            )
                            bi += 1
                            if bi < len(batches) - 1:
                                stg = stg_pool.tile([128, STG_PAIRS, W],
                                                    BF16, tag="stg")
            if timing:
                nc.sync.dma_start(dummy.ap()[:], wt[0:1, 0, 0:16])
    nc.compile()
    return nc


def _host_inputs(x: np.ndarray, weight: np.ndarray):
    """Pack the shifted + padded per-channel images into the row-interleaved
    partition layout (bf16), and build the 6 block tap matrices."""
    xv = np.asarray(x, dtype=np.float32).reshape(
        N_CORES, B_PER_CORE, C, H, W)
    xp = np.zeros((N_CORES, B_PER_CORE, C, H + 2, W + 2), np.float32)
    xp[:, :, :, 1:H + 1, 1:W + 1] = xv
    for g in range(5):
        ch = (np.arange(C) % 5) == g
        xp[:, :, ch] = np.roll(xp[:, :, ch], (DYS[g], DXS[g]), axis=(3, 4))
    # [cores, 128, samples, slots, cols]: p<64 even rows, p>=64 odd rows
    full = np.empty((N_CORES, 128, B_PER_CORE, SLOTS, VP), np.float32)
    full[:, 0:64] = xp[:, :, :, 0::2, :].transpose(0, 2, 1, 3, 4)
    full[:, 64:128] = xp[:, :, :, 1::2, :].transpose(0, 2, 1, 3, 4)
    x_packed = full.astype(ml_dtypes.bfloat16)

    # block tap matrices [128(k), 6, 128(m)]; k = (row parity, ci),
    # m = (out-row parity, co); t = pass*3 + kw
    wk = np.asarray(weight, dtype=np.float32).transpose(1, 2, 3, 0)
    # wk[ci, kh, kw, co]
    wt_host = np.zeros((128, 6, 128), np.float32)
    for kw in range(3):
        # pass A: slot h/2 = padded rows (h, h+1)
        wt_host[0:64, kw, 0:64] = wk[:, 0, kw, :]        # row h   -> out h
        wt_host[64:128, kw, 0:64] = wk[:, 1, kw, :]      # row h+1 -> out h
        wt_host[64:128, kw, 64:128] = wk[:, 0, kw, :]    # row h+1 -> out h+1
        # pass B: slot h/2+1 = padded rows (h+2, h+3)
        wt_host[0:64, 3 + kw, 0:64] = wk[:, 2, kw, :]    # row h+2 -> out h
        wt_host[0:64, 3 + kw, 64:128] = wk[:, 1, kw, :]  # row h+2 -> out h+1
        wt_host[64:128, 3 + kw, 64:128] = wk[:, 2, kw, :]  # row h+3 -> h+1
    return x_packed, wt_host.astype(ml_dtypes.bfloat16)


_NC_CACHE = {}


def _get_nc(reps: int = 1):
    if reps not in _NC_CACHE:
        _NC_CACHE[reps] = build_kernel(reps)
    return _NC_CACHE[reps]


def kernel(x: np.ndarray, weight: np.ndarray) -> np.ndarray:
    x = np.asarray(x, dtype=np.float32)
    weight = np.asarray(weight, dtype=np.float32)
    x_packed, wt_host = _host_inputs(x, weight)
    nc = _get_nc(1)
    in_maps = [
        {"x": np.ascontiguousarray(x_packed[k]), "wt": wt_host}
        for k in range(N_CORES)
    ]
    res = run_bass_kernel_spmd(nc, in_maps, core_ids=list(range(N_CORES)))
    out = np.empty((B, C, H, W), np.float32)
    for k in range(N_CORES):
        r = np.asarray(res.results[k]["out"]).astype(np.float32)
        # r[s, par, co, hp, w] -> out rows 2*hp + par
        for s in range(B_PER_CORE):
            ov = out[k * B_PER_CORE + s].reshape(C, H // 2, 2, W)
            ov[:, :, 0, :] = r[s, 0]
            ov[:, :, 1, :] = r[s, 1]
        tail = np.asarray(res.results[k]["out_tail"]).astype(np.float32)
        out[k * B_PER_CORE + 1, :, H - 2, :] = tail[0:64]
        out[k * B_PER_CORE + 1, :, H - 1, :] = tail[64:128]
    return out


# revision 4
# speedup vs baseline: 1.4338x; 1.1558x over previous
"""Trainium2 Bass kernel for nn_DConv (shift-gather + 3x3 conv), 8 NeuronCores.

Math: the reference's per-channel torch.roll on the zero-padded image only
wraps into zero-pad rows/columns, so the op collapses to a 3x3 conv over a
host-pre-shifted, zero-padded image (the roll + pad are pure data layout,
applied while packing partitions on the host).

PE packing (the key trick): instead of batching two samples into one
block-masked matmul (50% useful PE occupancy), pack TWO IMAGE ROWS into the
contraction dim and TWO OUTPUT ROWS into the output dim:

  K = 128 = 64 ci x {even row, odd row}   (partition p<64: ci=p, padded rows
      2j at slot j; p>=64: ci=p-64, padded rows 2j+1 at slot j)
  M = 128 = 64 co x {even out row, odd out row}

An output row pair (h, h+1), h even, needs padded rows h..h+3 = slots
h/2, h/2+1.  Six accumulating matmuls (2 slots x 3 kw taps) with fixed
128x128 block weights (3 of 4 blocks nonzero = 75% useful density) produce
both output rows for all 64 out-channels.  Per core (2 samples, processed
back to back) the PE streams 2 x 80 x 6 x 160 = 153,600 moving columns
vs 230,400 for the sample-pair scheme -- a 1.5x matmul-time cut.

Samples are data-parallel over batch: 2 per core, computed sequentially on
the full 128-partition array.

HBM output layout is (sample, row-parity, co, h/2, w) so each partition's
store rows are contiguous in HBM (multi-row >=512B descriptors at full DMA
rate); the host de-interleaves parity when unpacking (free).

Backpressure control: each sample gets a full-size SBUF staging buffer, so
PSUM->SBUF copies never wait on store DMAs (store transfers queue behind
the big input-band loads on the shared DMA engines; with rotating staging
that dependency chain stalled the PE mid-kernel and reset the p-state).

Dtype: bfloat16 operands, fp32 PSUM accumulation; output stored as bf16
(final two row pairs as an fp32 sidecar to keep the end-of-kernel critical
path short) and upcast on the host.

Schedule: PE ramps on dummy matmuls over a zeroed tile from ~t=0 while
weights + first image rows arrive; sample 0 uses a soft-start chunk plan;
loads are large contiguous row-band DMAs alternating the two HWDGE rings;
mid-kernel stores run on SWDGE (gpsimd) so they can't head-of-line-block
the loads; the three final small stores use the then-idle scalar ring and
the sidecar the sync ring.
"""
import numpy as np
import ml_dtypes

from concourse import bacc, tile, mybir
from concourse.bass_utils import run_bass_kernel_spmd

# problem shape (hardcoded per contract)
B, C, H, W = 16, 64, 160, 160
N_CORES = 8
B_PER_CORE = B // N_CORES  # 2
VP = W + 2                 # padded col pitch 162
SLOTS = (H + 2) // 2       # 81 row-pair slots (padded rows 0..161)

BF16 = mybir.dt.bfloat16
F32 = mybir.dt.float32

# shift table: group g = ci % 5
DXS = [0, 1, 0, -1, 0]
DYS = [0, 0, 1, 0, -1]

# chunk plans: (pair_base, n_pairs) per PSUM chunk (n_pairs*160 <= 512).
# sample 0 soft-starts so compute begins on the first loaded slots; sample 1
# tapers to 2-pair chunks for a short store tail, with the last chunk going
# to the f32 sidecar.
S0_CHUNKS = [(0, 1), (1, 1), (2, 2), (4, 2)] + \
    [(6 + 3 * i, 3) for i in range(24)] + [(78, 2)]
S1_CHUNKS = [(3 * i, 3) for i in range(24)] + \
    [(72, 2), (74, 2), (76, 2), (78, 2)]
# store-batch boundaries in pairs (must align with chunk ends)
S0_BATCH = [0, 18, 36, 54, 72, 80]
S1_BATCH = [0, 18, 36, 54, 72, 74, 76, 78]   # pairs 78-79 via sidecar
# row-band loads (slot ranges) per sample, alternating HWDGE rings
S0_BANDS = [(0, 4), (4, 14), (14, 40), (40, 81)]
S1_BANDS = [(0, 27), (27, 54), (54, 81)]

WARMUP_MMS = 14            # dummy 256-col matmuls ramping the PE from ~t=0


def build_kernel(reps: int = 1, timing: bool = False):
    nc = bacc.Bacc("TRN2", target_bir_lowering=False, debug=False,
                   num_devices=N_CORES)
    wt_dram = nc.dram_tensor("wt", [128, 6, 128], BF16,
                             kind="ExternalInput")
    if timing:
        # timing-only variant: big tensors stay in device DRAM (uninitialised
        # garbage is fine for timing) so per-call host<->device transfer is
        # tiny and wall-clock noise is dominated by the fixed RTT only.
        x_dram = nc.dram_tensor("x", [128, B_PER_CORE, SLOTS, VP], BF16)
        out_dram = nc.dram_tensor(
            "out", [B_PER_CORE, 2, C, H // 2, W], BF16)
        tail_dram = nc.dram_tensor("out_tail", [128, 2, W], F32)
        dummy = nc.dram_tensor("t_dummy", [1, 16], BF16,
                               kind="ExternalOutput")
    else:
        x_dram = nc.dram_tensor("x", [128, B_PER_CORE, SLOTS, VP], BF16,
                                kind="ExternalInput")
        out_dram = nc.dram_tensor(
            "out", [B_PER_CORE, 2, C, H // 2, W], BF16,
            kind="ExternalOutput")
        tail_dram = nc.dram_tensor("out_tail", [128, 2, W], F32,
                                   kind="ExternalOutput")
    x_ap = x_dram.ap()
    # partition view: p = parity*64 + co
    out_view = out_dram.ap().rearrange("s par co hp w -> s (par co) hp w")

    with tile.TileContext(nc) as tc:
        with (
            tc.tile_pool(name="wpool", bufs=1) as wpool,
            tc.tile_pool(name="psum", bufs=6, space="PSUM") as psum_pool,
        ):
            # tiny zero tile for PE warmup: dummy matmuls start at ~t=0 with
            # no input dependency, burning the PE p-state ramp while the
            # weights and first image rows arrive
            wz = wpool.tile([128, 256], BF16, tag="wz")
            nc.vector.memset(wz[:].bitcast(F32), 0.0)

            wt = wpool.tile([128, 6, 128], BF16, tag="wt")
            nc.sync.dma_start(wt[:], wt_dram.ap()[:])
            tailbuf = wpool.tile([128, 2, W], F32, tag="tail")
            xs0 = wpool.tile([128, SLOTS, VP], BF16, tag="xs0")
            xs1 = wpool.tile([128, SLOTS, VP], BF16, tag="xs1")
            xs = [xs0, xs1]
            # full-size staging per sample: stores never backpressure copies
            stg0 = wpool.tile([128, 80, W], BF16, tag="stg0")
            stg1 = wpool.tile([128, 78, W], BF16, tag="stg1")
            stg = [stg0, stg1]

            if WARMUP_MMS:
                psw = psum_pool.tile([128, 512], F32, tag="ps")
                for i in range(WARMUP_MMS):
                    nc.tensor.matmul(psw[:, 0:256], wz[:, 0:128], wz[:],
                                     start=(i == 0),
                                     stop=(i == WARMUP_MMS - 1))

            for _ in range(reps):
                # issue all row-band loads up front, alternating HWDGE rings
                qi = 0
                for s, bands in ((0, S0_BANDS), (1, S1_BANDS)):
                    for a, b in bands:
                        eng = nc.scalar if qi % 2 == 0 else nc.sync
                        eng.dma_start(xs[s][:, a:b, :], x_ap[:, s, a:b, :])
                        qi += 1

                for s, chunks, batches in (
                        (0, S0_CHUNKS, S0_BATCH), (1, S1_CHUNKS, S1_BATCH)):
                    bi = 0       # current batch index
                    last = len(chunks) - 1
                    for ci_, (jp, npair) in enumerate(chunks):
                        n_out = W * npair
                        ps = psum_pool.tile([128, 512], F32, tag="ps")
                        ps_view = ps[:, 0:n_out].rearrange(
                            "p (r v) -> p r v", v=W)
                        for t in range(6):
                            pas, kw = divmod(t, 3)
                            nc.tensor.matmul(
                                ps_view[:],
                                wt[:, t, :],
                                xs[s][:, jp + pas:jp + pas + npair,
                                      kw:kw + W],
                                start=(t == 0),
                                stop=(t == 5),
                            )
                        if s == 1 and ci_ == last:
                            # final two row pairs: f32 copy + f32 sidecar
                            # store (host merges), keeping the end-of-kernel
                            # critical path as short as possible
                            nc.vector.tensor_copy(
                                tailbuf[:].rearrange("p n v -> p (n v)"),
                                ps[:, 0:n_out])
                            nc.sync.dma_start(tail_dram.ap()[:],
                                              tailbuf[:])
                            continue
                        nc.vector.tensor_copy(
                            stg[s][:, jp:jp + npair, :], ps_view[:])
                        if jp + npair == batches[bi + 1]:
                            # flush the batch: rows are hp-contiguous in HBM
                            # so each partition is one big descriptor.  SWDGE
                            # (gpsimd) keeps mid-kernel stores off the HWDGE
                            # load rings; the small final batches use the
                            # then-idle scalar ring.
                            b0, b1 = batches[bi], batches[bi + 1]
                            eng = (nc.scalar if s == 1 and b0 >= 72
                                   else nc.gpsimd)
                            eng.dma_start(
                                out_view[s, :, b0:b1, :],
                                stg[s][:, b0:b1, :],
                            )
                            bi += 1
            if timing:
                nc.sync.dma_start(dummy.ap()[:], wt[0:1, 0, 0:16])
    nc.compile()
    return nc


def _host_inputs(x: np.ndarray, weight: np.ndarray):
    """Pack the shifted + padded per-channel images into the row-interleaved
    partition layout (bf16), and build the 6 block tap matrices."""
    xv = np.asarray(x, dtype=np.float32).reshape(
        N_CORES, B_PER_CORE, C, H, W)
    xp = np.zeros((N_CORES, B_PER_CORE, C, H + 2, W + 2), np.float32)
    xp[:, :, :, 1:H + 1, 1:W + 1] = xv
    for g in range(5):
        ch = (np.arange(C) % 5) == g
        xp[:, :, ch] = np.roll(xp[:, :, ch], (DYS[g], DXS[g]), axis=(3, 4))
    # [cores, 128, samples, slots, cols]: p<64 even rows, p>=64 odd rows
    full = np.empty((N_CORES, 128, B_PER_CORE, SLOTS, VP), np.float32)
    full[:, 0:64] = xp[:, :, :, 0::2, :].transpose(0, 2, 1, 3, 4)
    full[:, 64:128] = xp[:, :, :, 1::2, :].transpose(0, 2, 1, 3, 4)
    x_packed = full.astype(ml_dtypes.bfloat16)

    # block tap matrices [128(k), 6, 128(m)]; k = (row parity, ci),
    # m = (out-row parity, co); t = pass*3 + kw
    wk = np.asarray(weight, dtype=np.float32).transpose(1, 2, 3, 0)
    # wk[ci, kh, kw, co]
    wt_host = np.zeros((128, 6, 128), np.float32)
    for kw in range(3):
        # pass A: slot h/2 = padded rows (h, h+1)
        wt_host[0:64, kw, 0:64] = wk[:, 0, kw, :]        # row h   -> out h
        wt_host[64:128, kw, 0:64] = wk[:, 1, kw, :]      # row h+1 -> out h
        wt_host[64:128, kw, 64:128] = wk[:, 0, kw, :]    # row h+1 -> out h+1
        # pass B: slot h/2+1 = padded rows (h+2, h+3)
        wt_host[0:64, 3 + kw, 0:64] = wk[:, 2, kw, :]    # row h+2 -> out h
        wt_host[0:64, 3 + kw, 64:128] = wk[:, 1, kw, :]  # row h+2 -> out h+1
        wt_host[64:128, 3 + kw, 64:128] = wk[:, 2, kw, :]  # row h+3 -> h+1
    return x_packed, wt_host.astype(ml_dtypes.bfloat16)


_NC_CACHE = {}


def _get_nc(reps: int = 1):
    if reps not in _NC_CACHE:
        _NC_CACHE[reps] = build_kernel(reps)
    return _NC_CACHE[reps]


def kernel(x: np.ndarray, weight: np.ndarray) -> np.ndarray:
    x = np.asarray(x, dtype=np.float32)
    weight = np.asarray(weight, dtype=np.float32)
    x_packed, wt_host = _host_inputs(x, weight)
    nc = _get_nc(1)
    in_maps = [
        {"x": np.ascontiguousarray(x_packed[k]), "wt": wt_host}
        for k in range(N_CORES)
    ]
    res = run_bass_kernel_spmd(nc, in_maps, core_ids=list(range(N_CORES)))
    out = np.empty((B, C, H, W), np.float32)
    for k in range(N_CORES):
        r = np.asarray(res.results[k]["out"]).astype(np.float32)
        # r[s, par, co, hp, w] -> out rows 2*hp + par
        for s in range(B_PER_CORE):
            ov = out[k * B_PER_CORE + s].reshape(C, H // 2, 2, W)
            ov[:, :, 0, :] = r[s, 0]
            ov[:, :, 1, :] = r[s, 1]
        # sample 1 rows 156..159 come from the f32 sidecar (pairs 78, 79)
        tail = np.asarray(res.results[k]["out_tail"]).astype(np.float32)
        o1 = out[k * B_PER_CORE + 1]
        o1[:, 156, :] = tail[0:64, 0]
        o1[:, 157, :] = tail[64:128, 0]
        o1[:, 158, :] = tail[0:64, 1]
        o1[:, 159, :] = tail[64:128, 1]
    return out
